# revision 16
# baseline (speedup 1.0000x reference)
"""GAT (2-layer, single-head) Trainium2 Bass kernel, 8-core SPMD.

Design (v2): destination-major edge grid, no one-hot matmuls, no er gather.

  - Destination nodes are 1D-sharded: core c owns nodes [c*12500, (c+1)*12500).
  - Within a core, dst nodes are placed into 98 blocks of 128 (partition = dst)
    by clustering on the per-window in-degree vector (lexsort by (max, total)),
    minimizing slot padding.
  - Edge slots: dst d's incoming edges occupy row p(d) of its block's tile,
    one slot column per edge, grouped by source *window* (the int16 gather
    index limit splits the 100352-row feature table into 4 windows of 25088
    rows = 2 cores each, placement-independent).
  - Per-node feature table row = [el | feat(32) | er | pad] (64 f32, 256B).
    Each core computes rows for its own nodes (x @ W1aug) and the table is
    AllGathered; one dma_gather per 8 slot-columns (1024 indices - the SWDGE
    ucode cap; >1024 per op hangs the device) pulls feat[src] rows straight
    into the block tiles (window-major column order so ops batch across the
    group's blocks). Edges in window-overlap zones are rebalanced to the
    lighter adjacent window ("window flexing") to shrink padding.
  - Pad slots gather a sentinel row (el ~ -200 -> exp(lrelu) ~ 1e-17, feat=0).
  - Attention per block: t = el + er_d (DVE tensor_scalar add; er rides as a
    per-partition scalar - no er gather/broadcast needed); leaky on DVE;
    ex = Exp(t) on Act (NOTE: Act Lrelu gives wrong results on HW and each
    Lrelu<->Exp switch costs a 1.3us activation-table load - avoid);
    den = reduce(ex); num[d,f] = reduce_j(ex * feat) via a broadcast
    tensor_tensor and a transposed-view reduce on DVE;
    h = num/den + bias (+relu). Layer-2 table rows are produced inline per
    block (PE transpose + matmul) and the phase repeats.
  - Output rows are in placement order; the host inverts the permutation.
"""

import numpy as np

N = 100000
E = 1600000
F = 128
H = 32
NCORES = 8
NPC = N // NCORES          # 12500 dst nodes per core
P = 128
NB = (NPC + P - 1) // P    # 98 blocks
NPCP = NB * P              # 12544 padded nodes per core
TROWS = NCORES * NPCP      # 100352 table rows
TW = 64                    # table row: [el, feat(32), er, pad] = 256B f32
ELCOL = 0
FC0, FC1 = 1, 1 + H        # feat cols [1, 33)
ERCOL = 1 + H              # 33
NWIN = 4
WROWS = TROWS // NWIN      # 25088 rows per gather window (= 2 cores)
SENT = 12500               # sentinel row, window-relative (core 2w's 1st pad)
NPAD = NPCP - NPC          # 44 pad rows per core
GB = 5                     # blocks per gather/compute group
OPCOLS = 8                 # max slot-columns per dma_gather op (1024 idxs)
SCRATCH = 16384            # SWDGE descriptor ring (default): 1024 descs

_cache = {}


def _plan(R):
    """Column layout shared by host prep and program build.

    R: [NB, NWIN] slot-columns per (block, window).
    Returns (C, colbase, groups); groups[g] = (gstart, gcols, ops, blocks):
      ops    = [(w, lc0, ncols)] gather runs, local to the group tile
      blocks = [(b, dtot, ranges)] with ranges = [(lc0, D)] per window.
    """
    colbase = np.zeros((NB, NWIN), np.int64)
    groups = []
    c = 0
    for g in range(0, NB, GB):
        bs = list(range(g, min(g + GB, NB)))
        gstart = c
        ops = []
        ranges = {b: [] for b in bs}
        for w in range(NWIN):
            r0 = c
            for b in bs:
                colbase[b, w] = c
                ranges[b].append((c - gstart, int(R[b, w])))
                c += int(R[b, w])
            ops.append((w, r0 - gstart, c - r0))
        blocks = [(b, sum(d for (_o, d) in ranges[b]), ranges[b]) for b in bs]
        groups.append((gstart, c - gstart, ops, blocks))
    return c, colbase, groups


def _host_prep(x, src, dst, W1, al1, ar1, b1, W2, al2, ar2, b2):
    f32, i16 = np.float32, np.int16
    src = np.asarray(src).astype(np.int64)
    dst = np.asarray(dst).astype(np.int64)

    srccore = src // NPC
    w_e = srccore // 2                      # window of each edge's src row
    dcore = dst // NPC
    dloc = dst % NPC

    # --- placement per core: cluster dst nodes by per-window in-degree ---
    orders = []        # per core: position -> node-local-id (12500 entries)
    pos_of = np.zeros((NCORES, NPC), np.int64)
    deg_all = np.zeros((NCORES, NPCP, NWIN), np.int64)
    np.add.at(deg_all, (dcore, dloc, w_e), 1)
    for c in range(NCORES):
        deg = deg_all[c, :NPC]
        order = np.lexsort((deg.sum(1), deg.max(1)))
        orders.append(order)
        pos_of[c, order] = np.arange(NPC)

    # --- source table rows (depend on src core's placement) ---
    trow = srccore * NPCP + pos_of[srccore, src % NPC]
    pos_e = pos_of[dcore, dloc]

    # --- window flexing: gather windows are 32768 rows but spaced 25088
    # apart, so rows in [w*25088, w*25088+7680) are also addressable from
    # window w-1. Move such edges down-window to balance per-node degrees.
    node_e = dcore * NPCP + pos_e
    dual = (w_e >= 1) & ((trow - w_e * WROWS) < (32768 - WROWS))
    nodedeg = np.zeros((NCORES * NPCP, NWIN), np.int64)
    np.add.at(nodedeg, (node_e, w_e), 1)
    mov = np.zeros((NCORES * NPCP, NWIN), np.int64)
    np.add.at(mov, (node_e[dual], w_e[dual]), 1)
    kmove = np.zeros((NCORES * NPCP, NWIN), np.int64)
    for _ in range(2):
        for w in range(1, NWIN):
            k = np.clip((nodedeg[:, w] - nodedeg[:, w - 1] + 1) // 2,
                        0, mov[:, w] - kmove[:, w])
            kmove[:, w] += k
            nodedeg[:, w] -= k
            nodedeg[:, w - 1] += k
    # rank dual edges within (node, basew); move those with rank < kmove
    dkey = node_e * NWIN + w_e
    dkey_d = dkey[dual]
    od = np.argsort(dkey_d, kind="stable")
    ksd = dkey_d[od]
    nr = np.empty(len(ksd), bool)
    if len(ksd):
        nr[0] = True
        nr[1:] = ksd[1:] != ksd[:-1]
        st = np.nonzero(nr)[0]
        ri = np.cumsum(nr) - 1
        drank_s = np.arange(len(ksd)) - st[ri]
        drank = np.empty(len(ksd), np.int64)
        drank[od] = drank_s
        moved = drank < kmove[node_e[dual], w_e[dual]]
        wd = w_e[dual].copy()
        wd[moved] -= 1
        w_e = w_e.copy()
        w_e[dual] = wd

    # --- per-core R and shared max (post-flex degrees) ---
    degp = np.zeros((NCORES, NPCP, NWIN), np.int64)
    np.add.at(degp, (dcore, pos_e, w_e), 1)
    R = degp.reshape(NCORES, NB, P, NWIN).max(axis=2)   # [cores, NB, NWIN]
    R = np.maximum(R.max(axis=0), 1)                    # shared [NB, NWIN]
    C, colbase, groups = _plan(R)

    # --- per-edge slot assignment ---
    p_e = pos_e % P
    b_e = pos_e // P
    # rank within (core, node, window)
    key = ((dcore * NPCP + pos_e) * NWIN + w_e)
    order_e = np.argsort(key, kind="stable")
    ks = key[order_e]
    runstart = np.zeros(len(ks), np.int64)
    newrun = np.empty(len(ks), bool)
    newrun[0] = True
    newrun[1:] = ks[1:] != ks[:-1]
    runidx = np.cumsum(newrun) - 1
    starts = np.nonzero(newrun)[0]
    rank_sorted = np.arange(len(ks)) - starts[runidx]
    rank = np.empty(len(ks), np.int64)
    rank[order_e] = rank_sorted

    col = colbase[b_e, w_e] + rank
    fpos = col * P + p_e                                # per-core flat slot
    rel = (trow - w_e * WROWS).astype(i16)

    fidx = np.full((NCORES, 16, C * 8), SENT, i16)
    flat = dcore * (16 * C * 8) + (fpos % 16) * (C * 8) + fpos // 16
    fidx.reshape(-1)[flat] = rel
    fidx = np.tile(fidx, (1, 8, 1))                     # [NCORES, 128, C*8]

    def aug(W, al, ar):
        Wa = np.zeros((W.shape[0], TW), f32)
        Wa[:, FC0:FC1] = W
        Wa[:, ELCOL] = W @ al
        Wa[:, ERCOL] = W @ ar
        return Wa

    W1a = aug(np.asarray(W1, f32), np.asarray(al1, f32), np.asarray(ar1, f32))
    W2a = aug(np.asarray(W2, f32), np.asarray(al2, f32), np.asarray(ar2, f32))
    b1r = np.tile(np.asarray(b1, f32)[None, :], (P, 1))
    b2r = np.tile(np.asarray(b2, f32)[None, :], (P, 1))
    elm = np.zeros((P, 1), f32)
    elm[P - NPAD:, 0] = -200.0          # pad-row el offset (last block only)

    x = np.asarray(x, f32)
    xsT = np.zeros((NCORES, F, NPCP), f32)
    for c in range(NCORES):
        xsT[c, :, :NPC] = x[c * NPC:(c + 1) * NPC][orders[c]].T

    in_maps = []
    for c in range(NCORES):
        in_maps.append({
            "xsT": xsT[c], "W1a": W1a, "W2a": W2a, "b1r": b1r, "b2r": b2r,
            "elm": elm, "fidx": fidx[c],
        })
    R_key = tuple(int(v) for v in R.reshape(-1))
    return in_maps, R_key, orders


def _build_program(R_key, single=False):
    import concourse.bacc as bacc
    import concourse.mybir as mybir
    import concourse.tile as tile
    from concourse.masks import make_identity

    dt = mybir.dt
    AF = mybir.ActivationFunctionType
    ALU = mybir.AluOpType
    AX = mybir.AxisListType

    R = np.asarray(R_key, np.int64).reshape(NB, NWIN)
    C, colbase, groups = _plan(R)
    ncores = 1 if single else NCORES

    nc = bacc.Bacc("TRN2", target_bir_lowering=False, debug=False,
                   num_devices=ncores, num_swdge_queues=4,
                   dynamic_dma_scratch_size=SCRATCH)

    xsT = nc.dram_tensor("xsT", [F, NPCP], dt.float32, kind="ExternalInput")
    W1a = nc.dram_tensor("W1a", [F, TW], dt.float32, kind="ExternalInput")
    W2a = nc.dram_tensor("W2a", [H, TW], dt.float32, kind="ExternalInput")
    b1r = nc.dram_tensor("b1r", [P, H], dt.float32, kind="ExternalInput")
    b2r = nc.dram_tensor("b2r", [P, H], dt.float32, kind="ExternalInput")
    elm = nc.dram_tensor("elm", [P, 1], dt.float32, kind="ExternalInput")
    fidx = nc.dram_tensor("fidx", [P, C * 8], dt.int16, kind="ExternalInput")
    out_ext = nc.dram_tensor("out", [NPCP, H], dt.float32,
                             kind="ExternalOutput")

    qn_state = [0]

    def qn():
        qn_state[0] = (qn_state[0] + 1) % 4
        return qn_state[0]

    with tile.TileContext(nc) as tc:
        with (
            tc.tile_pool(name="const", bufs=1) as const,
            tc.tile_pool(name="prod", bufs=4) as prod,
            tc.tile_pool(name="gath", bufs=2) as gpool,
            tc.tile_pool(name="fxp", bufs=2) as fxp,
            tc.tile_pool(name="tex", bufs=6) as tex,
            tc.tile_pool(name="pr", bufs=3) as prp,
            tc.tile_pool(name="epi", bufs=8) as epi,
            tc.tile_pool(name="ps", bufs=4, space="PSUM") as psumt,
            tc.tile_pool(name="dram", bufs=1, space="DRAM") as dram,
        ):
            W1a_sb = const.tile([F, TW], dt.float32)
            nc.sync.dma_start(out=W1a_sb[:], in_=W1a[:])
            W2a_sb = const.tile([H, TW], dt.float32)
            nc.sync.dma_start(out=W2a_sb[:], in_=W2a[:])
            b1r_sb = const.tile([P, H], dt.float32)
            nc.sync.dma_start(out=b1r_sb[:], in_=b1r[:])
            b2r_sb = const.tile([P, H], dt.float32)
            nc.sync.dma_start(out=b2r_sb[:], in_=b2r[:])
            ident = const.tile([P, P], dt.float32)
            make_identity(nc, ident[:])
            elm_sb = const.tile([P, 1], dt.float32)
            nc.sync.dma_start(out=elm_sb[:], in_=elm[:])
            er1_sb = const.tile([P, NB], dt.float32, tag="er1")
            er2_sb = const.tile([P, NB], dt.float32, tag="er2")

            feat1_s = dram.tile([NPCP, TW], dt.float32)
            feat1_f = dram.tile([TROWS, TW], dt.float32, addr_space="Shared")
            feat2_s = dram.tile([NPCP, TW], dt.float32)
            feat2_f = dram.tile([TROWS, TW], dt.float32, addr_space="Shared")

            def finish_row(fsb, er_sb, b, feat_s):
                if b == NB - 1:
                    nc.vector.tensor_tensor(
                        out=fsb[:, ELCOL:ELCOL + 1],
                        in0=fsb[:, ELCOL:ELCOL + 1], in1=elm_sb[:],
                        op=mybir.AluOpType.add)
                nc.vector.tensor_copy(out=er_sb[:, b:b + 1],
                                      in_=fsb[:, ERCOL:ERCOL + 1])
                nc.sync.dma_start(out=feat_s[b * P:(b + 1) * P, :], in_=fsb[:])

            # ---- layer-1 table production ----
            for b in range(NB):
                xt = prod.tile([F, P], dt.float32, tag="xt")
                nc.sync.dma_start(out=xt[:], in_=xsT[:, b * P:(b + 1) * P])
                pmm = psumt.tile([P, TW], dt.float32, tag="pmm")
                nc.tensor.matmul(out=pmm[:], lhsT=xt[:], rhs=W1a_sb[:],
                                 start=True, stop=True)
                fsb = prod.tile([P, TW], dt.float32, tag="fsb")
                nc.vector.tensor_copy(out=fsb[:], in_=pmm[:])
                finish_row(fsb, er1_sb, b, feat1_s)

            def allgather(src_t, dst_t):
                if single:
                    nc.sync.dma_start(out=dst_t[0:NPCP, :], in_=src_t[:])
                else:
                    nc.gpsimd.collective_compute(
                        "AllGather", mybir.AluOpType.bypass,
                        replica_groups=[list(range(NCORES))],
                        ins=[src_t[:]], outs=[dst_t[:]],
                    )

            allgather(feat1_s, feat1_f)

            # ---- edge phase ----
            def edge_phase(feat_f, er_sb, bias_sb, relu, writer):
                for (gstart, gcols, ops, blocks) in groups:
                    fx = fxp.tile([P, gcols * 8], dt.int16, tag="fx")
                    nc.sync.dma_start(
                        out=fx[:], in_=fidx[:, gstart * 8:(gstart + gcols) * 8])
                    T = gpool.tile([P, gcols * TW], dt.float32, tag="T")
                    Tv = T[:].rearrange("p (c e) -> p c e", e=TW)
                    for (w, lc0, ncols) in ops:
                        whi = min(w * WROWS + 32768, TROWS)
                        for off in range(0, ncols, OPCOLS):
                            take = min(OPCOLS, ncols - off)
                            nc.gpsimd.dma_gather(
                                out_ap=Tv[:, lc0 + off:lc0 + off + take, :],
                                in_ap=feat_f[w * WROWS:whi, :],
                                idxs_ap=fx[:, (lc0 + off) * 8:
                                           (lc0 + off + take) * 8],
                                num_idxs=take * P, num_idxs_reg=take * P,
                                elem_size=TW, queue_num=qn(),
                            )
                    for (b, dtot, ranges) in blocks:
                        t = tex.tile([P, dtot], dt.float32, tag="t")
                        o = 0
                        for (lc0, D) in ranges:
                            nc.vector.tensor_scalar_add(
                                out=t[:, o:o + D], in0=Tv[:, lc0:lc0 + D, ELCOL],
                                scalar1=er_sb[:, b:b + 1])
                            o += D
                        et = tex.tile([P, dtot], dt.float32, tag="et")
                        nc.vector.tensor_scalar_mul(out=et[:], in0=t[:],
                                                    scalar1=0.2)
                        nc.vector.tensor_tensor(out=t[:], in0=t[:], in1=et[:],
                                                op=ALU.max)
                        ex = tex.tile([P, dtot], dt.float32, tag="ex")
                        nc.scalar.activation(out=ex[:], in_=t[:], func=AF.Exp)
                        den = epi.tile([P, 1], dt.float32, tag="den")
                        nc.vector.tensor_reduce(out=den[:], in_=ex[:],
                                                axis=AX.X, op=ALU.add)
                        pr = prp.tile([P, dtot * H], dt.float32, tag="pr")
                        prv = pr[:].rearrange("p (c f) -> p c f", f=H)
                        o = 0
                        for (lc0, D) in ranges:
                            exv = ex[:, o:o + D].rearrange(
                                "p (c u) -> p c u", u=1).broadcast_to((P, D, H))
                            nc.vector.tensor_tensor(
                                out=prv[:, o:o + D, :],
                                in0=Tv[:, lc0:lc0 + D, FC0:FC1],
                                in1=exv, op=ALU.mult)
                            o += D
                        num = epi.tile([P, H], dt.float32, tag="num")
                        nc.vector.tensor_reduce(
                            out=num[:], in_=pr[:].rearrange(
                                "p (c f) -> p f c", f=H),
                            axis=AX.X, op=ALU.add)
                        rec = epi.tile([P, 1], dt.float32, tag="rec")
                        nc.vector.reciprocal(out=rec[:], in_=den[:])
                        h = epi.tile([P, H], dt.float32, tag="h")
                        nc.vector.tensor_scalar_mul(out=h[:], in0=num[:],
                                                    scalar1=rec[:])
                        nc.vector.tensor_tensor(out=h[:], in0=h[:],
                                                in1=bias_sb[:], op=ALU.add)
                        if relu:
                            nc.scalar.activation(out=h[:], in_=h[:],
                                                 func=AF.Relu)
                        writer(b, h)

            def l1_writer(b, h):
                pt = psumt.tile([H, P], dt.float32, tag="pt")
                nc.tensor.transpose(out=pt[:], in_=h[:], identity=ident[:])
                hT = prod.tile([H, P], dt.float32, tag="hT")
                nc.vector.tensor_copy(out=hT[:], in_=pt[:])
                pmm2 = psumt.tile([P, TW], dt.float32, tag="pmm")
                nc.tensor.matmul(out=pmm2[:], lhsT=hT[:], rhs=W2a_sb[:],
                                 start=True, stop=True)
                f2 = prod.tile([P, TW], dt.float32, tag="fsb")
                nc.vector.tensor_copy(out=f2[:], in_=pmm2[:])
                finish_row(f2, er2_sb, b, feat2_s)

            edge_phase(feat1_f, er1_sb, b1r_sb, True, l1_writer)
            allgather(feat2_s, feat2_f)

            def l2_writer(b, h):
                nc.sync.dma_start(out=out_ext[b * P:(b + 1) * P, :], in_=h[:])

            edge_phase(feat2_f, er2_sb, b2r_sb, False, l2_writer)

    nc.compile()
    return nc


def _get_program(R_key, single=False):
    key = ("prog", R_key, single)
    if key not in _cache:
        _cache[key] = _build_program(R_key, single=single)
    return _cache[key]


def kernel(x, src, dst, W1, al1, ar1, b1, W2, al2, ar2, b2):
    from concourse.bass_utils import run_bass_kernel_spmd

    in_maps, R_key, orders = _host_prep(x, src, dst, W1, al1, ar1, b1,
                                        W2, al2, ar2, b2)
    nc = _get_program(R_key)
    res = run_bass_kernel_spmd(nc, in_maps, list(range(NCORES)))
    out = np.empty((N, H), np.float32)
    for c in range(NCORES):
        oc = np.asarray(res.results[c]["out"], np.float32)
        out[c * NPC + orders[c]] = oc[:NPC]
    return out


# revision 22
# speedup vs baseline: 1.0264x; 1.0264x over previous
"""GAT (2-layer, single-head) Trainium2 Bass kernel, 8-core SPMD.

Design (v2): destination-major edge grid, no one-hot matmuls, no er gather.

  - Destination nodes are 1D-sharded: core c owns nodes [c*12500, (c+1)*12500).
  - Within a core, dst nodes are placed into 98 blocks of 128 (partition = dst)
    by clustering on the per-window in-degree vector (lexsort by (max, total)),
    minimizing slot padding.
  - Edge slots: dst d's incoming edges occupy row p(d) of its block's tile,
    one slot column per edge, grouped by source *window* (the int16 gather
    index limit splits the 100352-row feature table into 4 windows of 25088
    rows = 2 cores each, placement-independent).
  - Per-node feature table row = [el | feat(32) | er | pad] (64 f32, 256B).
    Each core computes rows for its own nodes (x @ W1aug) and the table is
    AllGathered; one dma_gather per 8 slot-columns (1024 indices - the SWDGE
    ucode cap; >1024 per op hangs the device) pulls feat[src] rows straight
    into the block tiles (window-major column order so ops batch across the
    group's blocks). Edges in window-overlap zones are rebalanced to the
    lighter adjacent window ("window flexing") to shrink padding.
  - Pad slots gather a sentinel row (el ~ -200 -> exp(lrelu) ~ 1e-17, feat=0).
  - Attention per block: t = el + er_d (DVE tensor_scalar add; er rides as a
    per-partition scalar - no er gather/broadcast needed); leaky on DVE;
    ex = Exp(t) on Act (NOTE: Act Lrelu gives wrong results on HW and each
    Lrelu<->Exp switch costs a 1.3us activation-table load - avoid);
    den = reduce(ex); num[d,f] = reduce_j(ex * feat) via a broadcast
    tensor_tensor and a transposed-view reduce on DVE;
    h = num/den + bias (+relu). Layer-2 table rows are produced inline per
    block (PE transpose + matmul) and the phase repeats.
  - Output rows are in placement order; the host inverts the permutation.
"""

import numpy as np

N = 100000
E = 1600000
F = 128
H = 32
NCORES = 8
NPC = N // NCORES          # 12500 dst nodes per core
P = 128
NB = (NPC + P - 1) // P    # 98 blocks
NPCP = NB * P              # 12544 padded nodes per core
TROWS = NCORES * NPCP      # 100352 table rows
TW = 64                    # table row: [el, feat(32), er, pad] = 256B f32
ELCOL = 0
FC0, FC1 = 1, 1 + H        # feat cols [1, 33)
ERCOL = 1 + H              # 33
NWIN = 4
WROWS = TROWS // NWIN      # 25088 rows per gather window (= 2 cores)
SENT = 12500               # sentinel row, window-relative (core 2w's 1st pad)
NPAD = NPCP - NPC          # 44 pad rows per core
GB = 5                     # blocks per gather/compute group
OPCOLS = 8                 # max slot-columns per dma_gather op (1024 idxs)
SCRATCH = 16384            # SWDGE descriptor ring (default): 1024 descs

_cache = {}


def _plan(R):
    """Column layout shared by host prep and program build.

    R: [NB, NWIN] slot-columns per (block, window).
    Returns (C, colbase, groups); groups[g] = (gstart, gcols, ops, blocks):
      ops    = [(w, lc0, ncols)] gather runs, local to the group tile
      blocks = [(b, dtot, ranges)] with ranges = [(lc0, D)] per window.
    """
    colbase = np.zeros((NB, NWIN), np.int64)
    groups = []
    c = 0
    for g in range(0, NB, GB):
        bs = list(range(g, min(g + GB, NB)))
        gstart = c
        ops = []
        ranges = {b: [] for b in bs}
        for w in range(NWIN):
            r0 = c
            for b in bs:
                colbase[b, w] = c
                ranges[b].append((c - gstart, int(R[b, w])))
                c += int(R[b, w])
            ops.append((w, r0 - gstart, c - r0))
        blocks = [(b, sum(d for (_o, d) in ranges[b]), ranges[b]) for b in bs]
        groups.append((gstart, c - gstart, ops, blocks))
    return c, colbase, groups


def _host_prep(x, src, dst, W1, al1, ar1, b1, W2, al2, ar2, b2):
    f32, i16 = np.float32, np.int16
    src = np.asarray(src).astype(np.int64)
    dst = np.asarray(dst).astype(np.int64)

    srccore = src // NPC
    w_e = srccore // 2                      # window of each edge's src row
    dcore = dst // NPC
    dloc = dst % NPC

    # --- placement per core: cluster dst nodes by per-window in-degree ---
    orders = []        # per core: position -> node-local-id (12500 entries)
    pos_of = np.zeros((NCORES, NPC), np.int64)
    deg_all = np.zeros((NCORES, NPCP, NWIN), np.int64)
    np.add.at(deg_all, (dcore, dloc, w_e), 1)
    for c in range(NCORES):
        deg = deg_all[c, :NPC]
        order = np.lexsort((deg.sum(1), deg.max(1)))
        orders.append(order)
        pos_of[c, order] = np.arange(NPC)

    # --- source table rows (depend on src core's placement) ---
    trow = srccore * NPCP + pos_of[srccore, src % NPC]
    pos_e = pos_of[dcore, dloc]

    # --- window flexing: gather windows are 32768 rows but spaced 25088
    # apart, so rows in [w*25088, w*25088+7680) are also addressable from
    # window w-1. Move such edges down-window to balance per-node degrees.
    node_e = dcore * NPCP + pos_e
    dual = (w_e >= 1) & ((trow - w_e * WROWS) < (32768 - WROWS))
    nodedeg = np.zeros((NCORES * NPCP, NWIN), np.int64)
    np.add.at(nodedeg, (node_e, w_e), 1)
    mov = np.zeros((NCORES * NPCP, NWIN), np.int64)
    np.add.at(mov, (node_e[dual], w_e[dual]), 1)
    kmove = np.zeros((NCORES * NPCP, NWIN), np.int64)
    for _ in range(2):
        for w in range(1, NWIN):
            k = np.clip((nodedeg[:, w] - nodedeg[:, w - 1] + 1) // 2,
                        0, mov[:, w] - kmove[:, w])
            kmove[:, w] += k
            nodedeg[:, w] -= k
            nodedeg[:, w - 1] += k
    # rank dual edges within (node, basew); move those with rank < kmove
    dkey = node_e * NWIN + w_e
    dkey_d = dkey[dual]
    od = np.argsort(dkey_d, kind="stable")
    ksd = dkey_d[od]
    nr = np.empty(len(ksd), bool)
    if len(ksd):
        nr[0] = True
        nr[1:] = ksd[1:] != ksd[:-1]
        st = np.nonzero(nr)[0]
        ri = np.cumsum(nr) - 1
        drank_s = np.arange(len(ksd)) - st[ri]
        drank = np.empty(len(ksd), np.int64)
        drank[od] = drank_s
        moved = drank < kmove[node_e[dual], w_e[dual]]
        wd = w_e[dual].copy()
        wd[moved] -= 1
        w_e = w_e.copy()
        w_e[dual] = wd

    # --- per-core R and shared max (post-flex degrees) ---
    degp = np.zeros((NCORES, NPCP, NWIN), np.int64)
    np.add.at(degp, (dcore, pos_e, w_e), 1)
    R = degp.reshape(NCORES, NB, P, NWIN).max(axis=2)   # [cores, NB, NWIN]
    R = np.maximum(R.max(axis=0), 1)                    # shared [NB, NWIN]
    C, colbase, groups = _plan(R)

    # --- per-edge slot assignment ---
    p_e = pos_e % P
    b_e = pos_e // P
    # rank within (core, node, window)
    key = ((dcore * NPCP + pos_e) * NWIN + w_e)
    order_e = np.argsort(key, kind="stable")
    ks = key[order_e]
    runstart = np.zeros(len(ks), np.int64)
    newrun = np.empty(len(ks), bool)
    newrun[0] = True
    newrun[1:] = ks[1:] != ks[:-1]
    runidx = np.cumsum(newrun) - 1
    starts = np.nonzero(newrun)[0]
    rank_sorted = np.arange(len(ks)) - starts[runidx]
    rank = np.empty(len(ks), np.int64)
    rank[order_e] = rank_sorted

    col = colbase[b_e, w_e] + rank
    fpos = col * P + p_e                                # per-core flat slot
    rel = (trow - w_e * WROWS).astype(i16)

    fidx = np.full((NCORES, 16, C * 8), SENT, i16)
    flat = dcore * (16 * C * 8) + (fpos % 16) * (C * 8) + fpos // 16
    fidx.reshape(-1)[flat] = rel
    fidx = np.tile(fidx, (1, 8, 1))                     # [NCORES, 128, C*8]

    def aug(W, al, ar):
        Wa = np.zeros((W.shape[0], TW), f32)
        Wa[:, FC0:FC1] = W
        Wa[:, ELCOL] = W @ al
        Wa[:, ERCOL] = W @ ar
        return Wa

    W1a = aug(np.asarray(W1, f32), np.asarray(al1, f32), np.asarray(ar1, f32))
    W2a = aug(np.asarray(W2, f32), np.asarray(al2, f32), np.asarray(ar2, f32))
    b1r = np.tile(np.asarray(b1, f32)[None, :], (P, 1))
    b2r = np.tile(np.asarray(b2, f32)[None, :], (P, 1))
    elm = np.zeros((P, 1), f32)
    elm[P - NPAD:, 0] = -200.0          # pad-row el offset (last block only)

    x = np.asarray(x, f32)
    xsT = np.zeros((NCORES, F, NPCP), f32)
    for c in range(NCORES):
        xsT[c, :, :NPC] = x[c * NPC:(c + 1) * NPC][orders[c]].T

    in_maps = []
    for c in range(NCORES):
        in_maps.append({
            "xsT": xsT[c], "W1a": W1a, "W2a": W2a, "b1r": b1r, "b2r": b2r,
            "elm": elm, "fidx": fidx[c],
        })
    R_key = tuple(int(v) for v in R.reshape(-1))
    return in_maps, R_key, orders


def _build_program(R_key, single=False):
    import concourse.bacc as bacc
    import concourse.mybir as mybir
    import concourse.tile as tile
    from concourse.masks import make_identity

    dt = mybir.dt
    AF = mybir.ActivationFunctionType
    ALU = mybir.AluOpType
    AX = mybir.AxisListType

    R = np.asarray(R_key, np.int64).reshape(NB, NWIN)
    C, colbase, groups = _plan(R)
    ncores = 1 if single else NCORES

    nc = bacc.Bacc("TRN2", target_bir_lowering=False, debug=False,
                   num_devices=ncores, num_swdge_queues=4,
                   dynamic_dma_scratch_size=SCRATCH)

    xsT = nc.dram_tensor("xsT", [F, NPCP], dt.float32, kind="ExternalInput")
    W1a = nc.dram_tensor("W1a", [F, TW], dt.float32, kind="ExternalInput")
    W2a = nc.dram_tensor("W2a", [H, TW], dt.float32, kind="ExternalInput")
    b1r = nc.dram_tensor("b1r", [P, H], dt.float32, kind="ExternalInput")
    b2r = nc.dram_tensor("b2r", [P, H], dt.float32, kind="ExternalInput")
    elm = nc.dram_tensor("elm", [P, 1], dt.float32, kind="ExternalInput")
    fidx = nc.dram_tensor("fidx", [P, C * 8], dt.int16, kind="ExternalInput")
    out_ext = nc.dram_tensor("out", [NPCP, H], dt.float32,
                             kind="ExternalOutput")

    qn_state = [0]

    def qn():
        qn_state[0] = (qn_state[0] + 1) % 4
        return qn_state[0]

    with tile.TileContext(nc) as tc:
        with (
            tc.tile_pool(name="const", bufs=1) as const,
            tc.tile_pool(name="prod", bufs=4) as prod,
            tc.tile_pool(name="gath", bufs=3) as gpool,
            tc.tile_pool(name="tex", bufs=6) as tex,
            tc.tile_pool(name="pr", bufs=2) as prp,
            tc.tile_pool(name="epi", bufs=8) as epi,
            tc.tile_pool(name="ps", bufs=4, space="PSUM") as psumt,
            tc.tile_pool(name="dram", bufs=1, space="DRAM") as dram,
        ):
            W1a_sb = const.tile([F, TW], dt.float32)
            nc.sync.dma_start(out=W1a_sb[:], in_=W1a[:])
            W2a_sb = const.tile([H, TW], dt.float32)
            nc.sync.dma_start(out=W2a_sb[:], in_=W2a[:])
            b1r_sb = const.tile([P, H], dt.float32)
            nc.sync.dma_start(out=b1r_sb[:], in_=b1r[:])
            b2r_sb = const.tile([P, H], dt.float32)
            nc.sync.dma_start(out=b2r_sb[:], in_=b2r[:])
            ident = const.tile([P, P], dt.float32)
            make_identity(nc, ident[:])
            elm_sb = const.tile([P, 1], dt.float32)
            nc.sync.dma_start(out=elm_sb[:], in_=elm[:])
            fidx_sb = const.tile([P, C * 8], dt.int16)
            nc.sync.dma_start(out=fidx_sb[:], in_=fidx[:])
            er1_sb = const.tile([P, NB], dt.float32, tag="er1")
            er2_sb = const.tile([P, NB], dt.float32, tag="er2")

            feat1_s = dram.tile([NPCP, TW], dt.float32)
            feat1_f = dram.tile([TROWS, TW], dt.float32, addr_space="Shared")
            feat2_s = dram.tile([NPCP, TW], dt.float32)
            feat2_f = dram.tile([TROWS, TW], dt.float32, addr_space="Shared")

            def finish_row(fsb, er_sb, b, feat_s):
                if b == NB - 1:
                    nc.vector.tensor_tensor(
                        out=fsb[:, ELCOL:ELCOL + 1],
                        in0=fsb[:, ELCOL:ELCOL + 1], in1=elm_sb[:],
                        op=mybir.AluOpType.add)
                nc.vector.tensor_copy(out=er_sb[:, b:b + 1],
                                      in_=fsb[:, ERCOL:ERCOL + 1])
                nc.sync.dma_start(out=feat_s[b * P:(b + 1) * P, :], in_=fsb[:])

            # ---- layer-1 table production ----
            for b in range(NB):
                xt = prod.tile([F, P], dt.float32, tag="xt")
                nc.sync.dma_start(out=xt[:], in_=xsT[:, b * P:(b + 1) * P])
                pmm = psumt.tile([P, TW], dt.float32, tag="pmm")
                nc.tensor.matmul(out=pmm[:], lhsT=xt[:], rhs=W1a_sb[:],
                                 start=True, stop=True)
                fsb = prod.tile([P, TW], dt.float32, tag="fsb")
                nc.vector.tensor_copy(out=fsb[:], in_=pmm[:])
                finish_row(fsb, er1_sb, b, feat1_s)

            def allgather(src_t, dst_t):
                if single:
                    nc.sync.dma_start(out=dst_t[0:NPCP, :], in_=src_t[:])
                else:
                    nc.gpsimd.collective_compute(
                        "AllGather", mybir.AluOpType.bypass,
                        replica_groups=[list(range(NCORES))],
                        ins=[src_t[:]], outs=[dst_t[:]],
                    )

            allgather(feat1_s, feat1_f)

            # ---- edge phase ----
            def edge_phase(feat_f, er_sb, bias_sb, relu, writer):
                for (gstart, gcols, ops, blocks) in groups:
                    T = gpool.tile([P, gcols * TW], dt.float32, tag="T")
                    Tv = T[:].rearrange("p (c e) -> p c e", e=TW)
                    for (w, lc0, ncols) in ops:
                        whi = min(w * WROWS + 32768, TROWS)
                        for off in range(0, ncols, OPCOLS):
                            take = min(OPCOLS, ncols - off)
                            gc0 = gstart + lc0 + off
                            nc.gpsimd.dma_gather(
                                out_ap=Tv[:, lc0 + off:lc0 + off + take, :],
                                in_ap=feat_f[w * WROWS:whi, :],
                                idxs_ap=fidx_sb[:, gc0 * 8:(gc0 + take) * 8],
                                num_idxs=take * P, num_idxs_reg=take * P,
                                elem_size=TW, queue_num=qn(),
                            )
                    for (b, dtot, ranges) in blocks:
                        t = tex.tile([P, dtot], dt.float32, tag="t")
                        o = 0
                        for (lc0, D) in ranges:
                            nc.vector.tensor_scalar_add(
                                out=t[:, o:o + D], in0=Tv[:, lc0:lc0 + D, ELCOL],
                                scalar1=er_sb[:, b:b + 1])
                            o += D
                        et = tex.tile([P, dtot], dt.float32, tag="et")
                        nc.vector.tensor_scalar_mul(out=et[:], in0=t[:],
                                                    scalar1=0.2)
                        nc.vector.tensor_tensor(out=t[:], in0=t[:], in1=et[:],
                                                op=ALU.max)
                        ex = tex.tile([P, dtot], dt.float32, tag="ex")
                        nc.scalar.activation(out=ex[:], in_=t[:], func=AF.Exp)
                        den = epi.tile([P, 1], dt.float32, tag="den")
                        nc.vector.tensor_reduce(out=den[:], in_=ex[:],
                                                axis=AX.X, op=ALU.add)
                        pr = prp.tile([P, dtot * H], dt.float32, tag="pr")
                        prv = pr[:].rearrange("p (c f) -> p c f", f=H)
                        o = 0
                        for (lc0, D) in ranges:
                            exv = ex[:, o:o + D].rearrange(
                                "p (c u) -> p c u", u=1).broadcast_to((P, D, H))
                            nc.vector.tensor_tensor(
                                out=prv[:, o:o + D, :],
                                in0=Tv[:, lc0:lc0 + D, FC0:FC1],
                                in1=exv, op=ALU.mult)
                            o += D
                        num = epi.tile([P, H], dt.float32, tag="num")
                        nc.vector.tensor_reduce(
                            out=num[:], in_=pr[:].rearrange(
                                "p (c f) -> p f c", f=H),
                            axis=AX.X, op=ALU.add)
                        rec = epi.tile([P, 1], dt.float32, tag="rec")
                        nc.vector.reciprocal(out=rec[:], in_=den[:])
                        h = epi.tile([P, H], dt.float32, tag="h")
                        nc.vector.tensor_scalar_mul(out=h[:], in0=num[:],
                                                    scalar1=rec[:])
                        nc.vector.tensor_tensor(out=h[:], in0=h[:],
                                                in1=bias_sb[:], op=ALU.add)
                        if relu:
                            nc.scalar.activation(out=h[:], in_=h[:],
                                                 func=AF.Relu)
                        writer(b, h)

            def l1_writer(b, h):
                pt = psumt.tile([H, P], dt.float32, tag="pt")
                nc.tensor.transpose(out=pt[:], in_=h[:], identity=ident[:])
                hT = prod.tile([H, P], dt.float32, tag="hT")
                nc.vector.tensor_copy(out=hT[:], in_=pt[:])
                pmm2 = psumt.tile([P, TW], dt.float32, tag="pmm")
                nc.tensor.matmul(out=pmm2[:], lhsT=hT[:], rhs=W2a_sb[:],
                                 start=True, stop=True)
                f2 = prod.tile([P, TW], dt.float32, tag="fsb")
                nc.vector.tensor_copy(out=f2[:], in_=pmm2[:])
                finish_row(f2, er2_sb, b, feat2_s)

            edge_phase(feat1_f, er1_sb, b1r_sb, True, l1_writer)
            allgather(feat2_s, feat2_f)

            def l2_writer(b, h):
                nc.sync.dma_start(out=out_ext[b * P:(b + 1) * P, :], in_=h[:])

            edge_phase(feat2_f, er2_sb, b2r_sb, False, l2_writer)

    nc.compile()
    return nc


def _get_program(R_key, single=False):
    key = ("prog", R_key, single)
    if key not in _cache:
        _cache[key] = _build_program(R_key, single=single)
    return _cache[key]


def kernel(x, src, dst, W1, al1, ar1, b1, W2, al2, ar2, b2):
    from concourse.bass_utils import run_bass_kernel_spmd

    in_maps, R_key, orders = _host_prep(x, src, dst, W1, al1, ar1, b1,
                                        W2, al2, ar2, b2)
    nc = _get_program(R_key)
    res = run_bass_kernel_spmd(nc, in_maps, list(range(NCORES)))
    out = np.empty((N, H), np.float32)
    for c in range(NCORES):
        oc = np.asarray(res.results[c]["out"], np.float32)
        out[c * NPC + orders[c]] = oc[:NPC]
    return out
